# revision 1
# baseline (speedup 1.0000x reference)
"""DigitCaps dynamic-routing kernel for 8 Trainium2 NeuronCores.

Sharding:
  - s_j / squash / Wc: data-parallel over batch (B=256 -> 32 per core).
  - a_ij/b_ij update: sharded over ROUTES (1152 -> 144 per core), computed on the
    FULL batch, which needs v from every core -> AllGather(v) [32,160]->[256,160],
    and the resulting b-update slices are AllGather'd back [10,144]->[80,144].
    Two cheap AllGathers replace an AllReduce and cut the a-phase DVE/PE work 8x.

Algebra (never materialize u_hat = [B,R,C,O] 189MB):
  s_j[b,co] = sum_{(r,i)} xT[(r,i),b] * (c_ij[r,c]*W4[(r,i),co])    (K=9216 matmul)
  ab[r,c]   = (1/B) sum_{i,o} W4[(r,i),co] * G[(r,i),co],
  G         = sum_{all b} x[b,(r,i)] v[b,co]     (K=256 as two K=128 matmuls)
  then a DVE o-reduction and one PE matmul vs block-ones that i-reduces AND
  transposes, DMA'd straight into the collective bounce buffer.

Per-(r,i) partition mapping: chunk k covers routes 16k..16k+15, partition
p = 8*(r-16k)+i.  72 global chunks; each core owns local chunks 0..8.
"""

import sys
import numpy as np

sys.path.insert(0, "/opt/trn_rl_repo")

import concourse.bass as bass
import concourse.bacc as bacc
import concourse.mybir as mybir
import concourse.tile as tile
from concourse import bass_utils

F32 = mybir.dt.float32
F32R = mybir.dt.float32r
ALU = mybir.AluOpType
ACTF = mybir.ActivationFunctionType
AX = mybir.AxisListType

# s_j matmuls in float32r with 256-padded moving slots: 4 cycles/row -> 1.
# Costs ~1e-4 relative error end-to-end; False = full fp32 (~3e-6).
SJ_F32R = False

B, R, C, O, I = 256, 1152, 10, 16, 8
NCORES = 8
NB = B // NCORES            # 32 batch per core
RI = R * I                  # 9216 contraction dim
CO = C * O                  # 160 output cols
NCHUNK = RI // 128          # 72 chunks of 128 partitions
NGRP = 12                   # chunk groups (6 chunks each) for W4/Wc tiles
GC = NCHUNK // NGRP         # 6 chunks per group
GW = GC * CO                # 960 f32 per group
PSB = 512                   # PSUM bank size in f32
NLC = 9                     # local chunks per core (route shard)
RL = R // NCORES            # 144 local routes
WSPLIT = (1, 3, 4, 4)       # groups per w4/xt mega-tile (first alone -> early start)
SJP = 256 if SJ_F32R else CO   # w4/wc chunk-slot pitch (f32r needs moving dim >= 256)
SJDT = F32R if SJ_F32R else F32
GW2 = GC * SJP              # padded group width

_BUILT = None


def _warm_pe(tc, pools, src):
    """Tiny dummy matmul keyed on `src` so the PE HAM never sees an idle window."""
    nc = tc.nc
    wp = pools["warm"].tile([64, 2], F32, tag="warm", name="warm")
    nc.tensor.matmul(wp[:], lhsT=src[:, :64], rhs=src[:, :2])


def _squash(tc, pools, s_ps, scale):
    """v = sq*t/((1+sq)*sqrt(sq)) with t = scale*s, sq = t*t.  Returns SBUF tile [NB, CO]."""
    nc = tc.nc
    sb = pools["sb"]
    t = sb.tile([NB, CO], F32, tag="sq_t")
    sq = sb.tile([NB, CO], F32, tag="sq_sq")
    at = sb.tile([NB, CO], F32, tag="sq_at")
    num = sb.tile([NB, CO], F32, tag="sq_num")
    den = sb.tile([NB, CO], F32, tag="sq_den")
    rv = sb.tile([NB, CO], F32, tag="sq_rv")
    v = sb.tile([NB, CO], F32, tag="sq_v", bufs=2)
    s_ap = s_ps[:, :CO]
    # v = (t*|t|) / (1+t^2) with t = scale*s  (== sq*t/((1+sq)*sqrt(sq)) in reals)
    nc.scalar.activation(sq[:], s_ap, ACTF.Square, scale=scale)  # t^2
    nc.scalar.sqrt(at[:], sq[:])                              # |t|
    nc.scalar.mul(t[:], s_ap, scale)                          # t
    _warm_pe(tc, pools, sq)
    nc.vector.tensor_scalar_add(den[:], sq[:], 1.0)           # 1+t^2
    nc.vector.reciprocal(rv[:], den[:])
    nc.vector.tensor_mul(num[:], t[:], at[:])                 # t*|t|
    _warm_pe(tc, pools, num)
    nc.vector.tensor_mul(v[:], num[:], rv[:])
    _warm_pe(tc, pools, v)
    return v


def _sj_matmuls(tc, pools, xtv, rhsv):
    """s[b, co] = sum over all 72 chunks: xt_chunk^T @ rhs_chunk.  Returns PSUM tile.

    Under SJ_F32R the rhs slots are 256 wide (cols 160.. are zero pad) and the
    matmuls run in float32r; cols >= 160 of the PSUM tile are ignored."""
    nc = tc.nc
    width = rhsv(0).shape[-1]
    s_ps = pools["pbig"].tile([NB, SJP], F32, tag="gbig")
    for k in range(NCHUNK):
        nc.tensor.matmul(
            s_ps[:, :width],
            lhsT=xtv(k),
            rhs=rhsv(k),
            start=(k == 0),
            stop=(k == NCHUNK - 1),
        )
    return s_ps


def _ab_round(tc, pools, xr_t, w4s, v, bones, tag):
    """Full-batch route-sharded b-update round.

    AllGather v -> G over all 256 batches for our 9 local chunks -> P/o/i
    reductions -> AllGather the [10,144] slice -> return b-update [10,1152]."""
    nc = tc.nc
    pbig, psm, sb, dram = pools["pbig"], pools["psm"], pools["sb"], pools["dram"]

    ccv_in = dram.tile([NB, CO], F32, tag="ccvin", bufs=2, name=f"ccvin{tag}")
    ccv_out = dram.tile(
        [B, CO], F32, tag="ccvout", addr_space="Shared", bufs=2, name=f"ccvout{tag}"
    )
    nc.sync.dma_start(ccv_in[:], v[:])
    nc.gpsimd.collective_compute(
        "AllGather",
        ALU.bypass,
        replica_groups=[list(range(NCORES))],
        ins=[ccv_in[:].opt()],
        outs=[ccv_out[:].opt()],
    )
    vh = []
    for h in range(2):
        vt = sb.tile([128, CO], F32, tag=f"vh{h}", bufs=2, name=f"vh{h}")
        nc.sync.dma_start(vt[:], ccv_out[128 * h : 128 * h + 128, :])
        vh.append(vt)

    # G for 9 local chunks (full batch, K=256 as two accumulating K=128 matmuls)
    pr = sb.tile([128, NLC * C], F32, tag="pr", bufs=2, name=f"pr{tag}")
    for grp, (c0, nch) in enumerate(((0, 6), (6, 3))):
        g_ps = pbig.tile([128, 2 * PSB], F32, tag="gbig")
        for j in range(nch):
            off = PSB * (j // 3) + CO * (j % 3)
            for h in range(2):
                nc.tensor.matmul(
                    g_ps[:, off : off + CO],
                    lhsT=xr_t[h][:, 128 * (c0 + j) : 128 * (c0 + j) + 128],
                    rhs=vh[h][:],
                    start=(h == 0),
                    stop=(h == 1),
                )
        # P = (G/B) .* W4slice
        p_t = sb.tile([128, nch * CO], F32, tag="p", bufs=2)
        na = nch // 3
        g_view = g_ps[:].rearrange("p (a x) -> p a x", a=2)[:, :na, : 3 * CO].rearrange(
            "p a (s e) -> p a s e", s=3
        )
        w_view = w4s[:, CO * c0 : CO * (c0 + nch)].rearrange(
            "p (a s e) -> p a s e", a=na, s=3
        )
        p_view = p_t[:].rearrange("p (a s e) -> p a s e", a=na, s=3)
        nc.vector.scalar_tensor_tensor(
            p_view, g_view, 1.0 / B, w_view, ALU.mult, ALU.mult
        )
        # o-reduce into pr, c-major: free idx 9c + (c0+j)
        pr_view = (
            pr[:]
            .rearrange("p (c a) -> p c a", c=C)[:, :, c0 : c0 + nch]
            .transpose([0, 2, 1])
        )
        nc.vector.tensor_reduce(
            pr_view,
            p_t[:].rearrange("p (a o) -> p a o", o=O),
            axis=AX.X,
            op=ALU.add,
        )
    # i-reduce + transpose all 9 chunks in one matmul: out[9c+a, n] = ab[16a+n, c]
    q_ps = psm.tile([NLC * C, 16], F32, tag="sm", name="q_ps")
    nc.tensor.matmul(q_ps[:], lhsT=pr[:], rhs=bones[:])
    q_sb = sb.tile([NLC * C, 16], F32, tag="q_sb", bufs=2, name="q_sb")
    nc.scalar.copy(q_sb[:], q_ps[:])

    ccab_in = dram.tile([C, RL], F32, tag="ccabin", bufs=2, name=f"ccabin{tag}")
    ccab_out = dram.tile(
        [NCORES * C, RL], F32, tag="ccabout", addr_space="Shared", bufs=2,
        name=f"ccabout{tag}",
    )
    dst = ccab_in[:].rearrange("c (a n) -> c a n", a=NLC)
    nc.sync.dma_start(dst, q_sb[:])
    nc.gpsimd.collective_compute(
        "AllGather",
        ALU.bypass,
        replica_groups=[list(range(NCORES))],
        ins=[ccab_in[:].opt()],
        outs=[ccab_out[:].opt()],
    )
    # gather back as [10, 1152]: b[c, 144*rho + ri] = out[10*rho + c, ri]
    ar = sb.tile([C, R], F32, tag="ar", bufs=2, name=f"ar{tag}")
    src = ccab_out[:].rearrange("(rho c) ri -> c rho ri", c=C)
    nc.sync.dma_start(ar[:].rearrange("c (rho ri) -> c rho ri", rho=NCORES), src)
    return ar


def _softmax_cb_wc(tc, pools, bT, w4v, ident, selrepb, wc_t):
    """c = softmax(b) over routes; build Wc tiles = W4 .* broadcast(c)."""
    nc = tc.nc
    sb, psm = pools["sb"], pools["psm"]
    NRB = R // 128  # 9 route-blocks
    e = sb.tile([C, R], F32, tag="smx_e")
    ssum = sb.tile([C, 1], F32, tag="smx_s")
    sinv = sb.tile([C, 1], F32, tag="smx_si")
    nc.scalar.activation(e[:], bT[:], ACTF.Exp, accum_out=ssum[:])
    nc.vector.reciprocal(sinv[:], ssum[:])
    nc.vector.tensor_scalar_mul(e[:], e[:], sinv[:])  # e becomes c^T [10, 1152]
    # transpose c to route-major [128, 9*10]: col rb*10+c holds c[128rb+q, c]
    cr_all = sb.tile([128, NRB * C], F32, tag="cr_all", name="cr_all")
    for rb in range(NRB):
        tp = psm.tile([128, C], F32, tag="sm", name="tp")
        nc.tensor.transpose(tp[:], e[:, 128 * rb : 128 * rb + 128], ident[:C, :C])
        nc.scalar.copy(cr_all[:, C * rb : C * rb + C], tp[:])
    # replicate over i: for each s in 0..8, one matmul gives cb for chunks k=8rb+s:
    # out[p,(rb,c)] = cr_all[16s + p//8, (rb,c)].  Stored so col k*10+c = chunk k.
    cb_all = sb.tile([128, NCHUNK * C], F32, tag="cb_all", name="cb_all")
    cb_v = cb_all[:].rearrange("p (rb s c) -> p rb s c", s=8, c=C)
    for s in range(8):
        cb_ps = psm.tile([128, NRB * C], F32, tag="sm", name="cb_ps")
        nc.tensor.matmul(cb_ps[:], lhsT=selrepb[:, 128 * s : 128 * s + 128], rhs=cr_all[:])
        nc.scalar.copy(
            cb_v[:, :, s, :], cb_ps[:].rearrange("p (rb c) -> p rb c", c=C)
        )
    # per group: broadcast over o (step-0 AP), multiply into Wc
    for g in range(NGRP):
        cb_view = (
            cb_all[:, GC * C * g : GC * C * (g + 1)]
            .rearrange("p (j c) -> p j c", c=C)
            .unsqueeze(-1)
            .broadcast_to([128, GC, C, O])
        )
        w_view = w4v(g).rearrange("p (j c o) -> p j c o", j=GC, c=C)
        wc_view = (
            wc_t[g][:]
            .rearrange("p (j x) -> p j x", x=SJP)[:, :, :CO]
            .rearrange("p j (c o) -> p j c o", c=C)
        )
        nc.vector.tensor_mul(wc_view, w_view, cb_view)


def build():
    """Build the Bass module (one program, SPMD across 8 cores)."""
    nc = bacc.Bacc("TRN2", target_bir_lowering=False, debug=False, num_devices=NCORES)

    # chunk-major host layouts: free idx = 32k+b (xt), 160k+co (w4)
    d_xt = nc.dram_tensor("xt", [128, NCHUNK * NB], SJDT, kind="ExternalInput").ap()
    d_xr = nc.dram_tensor("xr", [B, RL * I], F32, kind="ExternalInput").ap()
    d_w4 = nc.dram_tensor("w4", [128, NCHUNK * CO], F32, kind="ExternalInput").ap()
    d_w4s = nc.dram_tensor("w4s", [128, NLC * CO], F32, kind="ExternalInput").ap()
    d_id = nc.dram_tensor("ident", [128, 128], F32, kind="ExternalInput").ap()
    d_sr = nc.dram_tensor("selrep", [128, 8 * 128], F32, kind="ExternalInput").ap()
    d_bo = nc.dram_tensor("bones", [128, 16], F32, kind="ExternalInput").ap()
    d_out = nc.dram_tensor("vout", [NB, CO], F32, kind="ExternalOutput").ap()

    with tile.TileContext(nc) as tc:
        with (
            tc.tile_pool(name="const", bufs=1) as const,
            tc.tile_pool(name="w4p", bufs=1) as w4p,
            tc.tile_pool(name="xtp", bufs=1) as xtp,
            tc.tile_pool(name="wcp", bufs=1) as wcp,
            tc.tile_pool(name="sb", bufs=1) as sb,
            tc.tile_pool(name="pbig", bufs=2, space="PSUM") as pbig,
            tc.tile_pool(name="psm", bufs=2, space="PSUM") as psm,
            tc.tile_pool(name="warm", bufs=1, space="PSUM") as warm,
            tc.tile_pool(name="dram", bufs=1, space="DRAM") as dram,
        ):
            pools = {"sb": sb, "pbig": pbig, "psm": psm, "warm": warm, "dram": dram}

            # warm up the collective engine + absorb launch skew with a throwaway
            # 32B AllGather, issued FIRST (on the otherwise-idle gpsimd queue) so
            # it completes during the input-DMA stream, long before the first
            # real collective.
            cw_in = dram.tile([1, 8], F32, tag="cwin", name="cw_in")
            cw_out = dram.tile(
                [8, 8], F32, tag="cwout", addr_space="Shared", name="cw_out"
            )
            cw_sb = sb.tile([1, 8], F32, tag="cw_sb")
            nc.gpsimd.memset(cw_sb[:], 0.0)
            nc.gpsimd.dma_start(cw_in[:], cw_sb[:])
            nc.gpsimd.collective_compute(
                "AllGather",
                ALU.bypass,
                replica_groups=[list(range(NCORES))],
                ins=[cw_in[:].opt()],
                outs=[cw_out[:].opt()],
            )

            # mega-tiles: WSPLIT groups each; first holds group 0 alone so its
            # DMA lands fast and s_j starts while the rest streams.  w4 on the
            # sync HWDGE queue, xt on the scalar HWDGE queue (both fast).
            w4_mt, xt_mt = [], []
            goff = 0
            for ti, ngr in enumerate(WSPLIT):
                wt = w4p.tile([128, ngr * GW], F32, tag=f"w4m{ti}", name=f"w4m{ti}")
                nc.sync.dma_start(wt[:], d_w4[:, GW * goff : GW * (goff + ngr)])
                xt_ = xtp.tile(
                    [128, ngr * GC * NB], SJDT, tag=f"xtm{ti}", name=f"xtm{ti}"
                )
                nc.scalar.dma_start(
                    xt_[:], d_xt[:, GC * NB * goff : GC * NB * (goff + ngr)]
                )
                w4_mt.append((goff, wt))
                xt_mt.append((goff, xt_))
                goff += ngr

            def w4v(g):
                for off, wt in reversed(w4_mt):
                    if g >= off:
                        return wt[:, GW * (g - off) : GW * (g - off + 1)]

            def xtv(k):
                g, j = k // GC, k % GC
                for off, xt_ in reversed(xt_mt):
                    if g >= off:
                        return xt_[:, NB * (GC * (g - off) + j) : NB * (GC * (g - off) + j) + NB]

            wc_t = []
            for g in range(NGRP):
                w = wcp.tile([128, GW2], SJDT, tag=f"wc_{g}", name=f"wc_{g}")
                if SJ_F32R:
                    nc.vector.memset(
                        w[:].rearrange("p (j x) -> p j x", x=SJP)[:, :, CO:].bitcast(F32),
                        0.0,
                    )
                wc_t.append(w)

            # route-shard inputs (needed from the first a-phase on)
            xr_t = []
            for h in range(2):
                xh = sb.tile([128, RL * I], F32, tag=f"xr{h}", name=f"xr{h}")
                nc.scalar.dma_start(xh[:], d_xr[128 * h : 128 * h + 128, :])
                xr_t.append(xh)
            w4s = sb.tile([128, NLC * CO], F32, tag="w4s")
            nc.scalar.dma_start(w4s[:], d_w4s[:])
            ident = const.tile([128, 128], F32, tag="ident")
            bones = const.tile([128, 16], F32, tag="bones")
            selrepb = const.tile([128, 8 * 128], F32, tag="selrepb")
            nc.scalar.dma_start(bones[:], d_bo[:])
            nc.scalar.dma_start(ident[:], d_id[:])
            nc.scalar.dma_start(selrepb[:], d_sr[:])

            # ---- iter 0: c uniform -> s = (X @ W4) / R
            # iter-0 s_j runs plain fp32 against compact W4 (it is DMA-paced anyway)
            xtv0 = (lambda k: xtv(k).bitcast(F32)) if SJ_F32R else xtv
            s0 = _sj_matmuls(tc, pools, xtv0, lambda k: w4v(k // GC)[:, CO * (k % GC) : CO * (k % GC) + CO])
            v0 = _squash(tc, pools, s0, 1.0 / R)
            bT1 = _ab_round(tc, pools, xr_t, w4s, v0, bones, tag=0)  # b was 0

            # ---- iter 1
            _softmax_cb_wc(tc, pools, bT1, w4v, ident, selrepb, wc_t)
            s1 = _sj_matmuls(tc, pools, xtv, lambda k: wc_t[k // GC][:, SJP * (k % GC) : SJP * (k % GC) + SJP])
            v1 = _squash(tc, pools, s1, 1.0)
            ar1 = _ab_round(tc, pools, xr_t, w4s, v1, bones, tag=1)
            bT2 = sb.tile([C, R], F32, tag="bT2")
            nc.vector.tensor_add(bT2[:], bT1[:], ar1[:])

            # ---- iter 2 (final)
            _softmax_cb_wc(tc, pools, bT2, w4v, ident, selrepb, wc_t)
            s2 = _sj_matmuls(tc, pools, xtv, lambda k: wc_t[k // GC][:, SJP * (k % GC) : SJP * (k % GC) + SJP])
            v2 = _squash(tc, pools, s2, 1.0)
            nc.sync.dma_start(d_out[:], v2[:])

    nc.compile()
    return nc


def _host_inputs(x, W):
    """Per-core input maps with pre-arranged layouts."""
    x = np.asarray(x, dtype=np.float32)
    W = np.asarray(W, dtype=np.float32)
    # W4[(r,i), (c,o)] chunk-major in the free dim: [128, 72*160]
    wf = np.ascontiguousarray(W.transpose(0, 3, 1, 2)).reshape(RI, CO)
    wfc = wf.reshape(NCHUNK, 128, CO)
    w4h = np.ascontiguousarray(wfc.transpose(1, 0, 2)).reshape(128, NCHUNK * CO)
    ident = np.eye(128, dtype=np.float32)
    # selrep[s][q, p] = 1 iff q == 16s + p//8, packed s-major in free dim
    selrep = np.zeros((8, 128, 128), dtype=np.float32)
    ss, qq, pp = np.meshgrid(np.arange(8), np.arange(128), np.arange(128), indexing="ij")
    selrep[qq == 16 * ss + pp // 8] = 1.0
    selrepb = np.ascontiguousarray(selrep.transpose(1, 0, 2)).reshape(128, 8 * 128)
    pq, jq = np.meshgrid(np.arange(128), np.arange(16), indexing="ij")
    bones = (pq // 8 == jq).astype(np.float32)
    in_maps = []
    for c in range(NCORES):
        xc = np.ascontiguousarray(x[NB * c : NB * (c + 1)].reshape(NB, RI))
        xt = np.ascontiguousarray(xc.T)  # [9216, 32]
        xth = np.ascontiguousarray(
            xt.reshape(NCHUNK, 128, NB).transpose(1, 0, 2)
        ).reshape(128, NCHUNK * NB)
        xrh = np.ascontiguousarray(x[:, RL * c : RL * (c + 1), :]).reshape(B, RL * I)
        w4sh = np.ascontiguousarray(
            wfc[NLC * c : NLC * (c + 1)].transpose(1, 0, 2)
        ).reshape(128, NLC * CO)
        in_maps.append(
            {
                "xt": xth,
                "xr": xrh,
                "w4": w4h,
                "w4s": w4sh,
                "ident": ident,
                "selrep": selrepb,
                "bones": bones,
            }
        )
    return in_maps


def kernel(x, W, trace=False):
    global _BUILT
    if _BUILT is None:
        _BUILT = build()
    nc = _BUILT
    in_maps = _host_inputs(x, W)
    res = bass_utils.run_bass_kernel_spmd(
        nc, in_maps, core_ids=list(range(NCORES)), trace=trace
    )
    v = np.concatenate([res.results[c]["vout"] for c in range(NCORES)], axis=0)
    out = v.reshape(B, C, O, 1)
    if trace:
        kernel.last_exec_time_ns = res.exec_time_ns
        kernel.last_results = res
    return out



# revision 4
# speedup vs baseline: 1.2358x; 1.2358x over previous
"""DigitCaps dynamic-routing kernel for 8 Trainium2 NeuronCores.

Sharding:
  - s_j / squash / Wc: data-parallel over batch (B=256 -> 32 per core).
  - a_ij/b_ij update: sharded over ROUTES (1152 -> 144 per core), computed on the
    FULL batch, which needs v from every core -> AllGather(v) [32,160]->[256,160],
    and the resulting b-update slices are AllGather'd back [10,144]->[80,144].
    Two cheap AllGathers replace an AllReduce and cut the a-phase DVE/PE work 8x.

Algebra (never materialize u_hat = [B,R,C,O] 189MB):
  s_j[b,co] = sum_{(r,i)} xT[(r,i),b] * (c_ij[r,c]*W4[(r,i),co])    (K=9216 matmul)
  ab[r,c]   = (1/B) sum_{i,o} W4[(r,i),co] * G[(r,i),co],
  G         = sum_{all b} x[b,(r,i)] v[b,co]     (K=256 as two K=128 matmuls)
  then a DVE o-reduction and one PE matmul vs block-ones that i-reduces AND
  transposes, DMA'd straight into the collective bounce buffer.

Precision: all big matmuls (s_j, G, cb expand, transposes) run in bf16
(1 cycle/row on PE vs fp32's 4); PSUM accumulation and the routing state
(b_ij, softmax sums, squash) stay fp32.  End-to-end rel err ~2e-3 vs the
2e-2 gate.

Layouts: w4/wc hold each chunk's 160 cols in (o,c) order so the Wc = W4*c
broadcast puts the stride-0 o-dim in the middle and keeps the last AP dim
packed -> DVE 2x mode.  squash consumes (o,c) PSUM but writes v strided in
(c,o), so the whole ab-phase (G, P, reductions, w4s) keeps (c,o) layout.

Per-(r,i) partition mapping: chunk k covers routes 16k..16k+15, partition
p = 8*(r-16k)+i.  72 global chunks; each core owns local chunks 0..8.

Engine budget: Scalar runs ONLY Exp (no activation-table swaps); squash and
all PSUM->SBUF copies run on DVE; scalar HWDGE ring carries the input
streams, sync HWDGE ring only the latency-critical collective bounce DMAs.
"""

import sys
import numpy as np

sys.path.insert(0, "/opt/trn_rl_repo")

import concourse.bass as bass
import concourse.bacc as bacc
import concourse.mybir as mybir
import concourse.tile as tile
from concourse import bass_utils

F32 = mybir.dt.float32
BF16 = mybir.dt.bfloat16
ALU = mybir.AluOpType
ACTF = mybir.ActivationFunctionType
AX = mybir.AxisListType

B, R, C, O, I = 256, 1152, 10, 16, 8
NCORES = 8
NB = B // NCORES            # 32 batch per core
RI = R * I                  # 9216 contraction dim
CO = C * O                  # 160 output cols
NCHUNK = RI // 128          # 72 chunks of 128 partitions
NGRP = 12                   # chunk groups (6 chunks each) for W4/Wc tiles
GC = NCHUNK // NGRP         # 6 chunks per group
GW = GC * CO                # 960 elems per group
PSB = 512                   # PSUM bank size in f32
NLC = 9                     # local chunks per core (route shard)
RL = R // NCORES            # 144 local routes
WSPLIT = (1, 3, 4, 4)       # groups per w4/xt mega-tile (first alone -> early start)

_BUILT = None


def _warm_pe(tc, pools, src):
    """Tiny dummy matmul keyed on `src` so the PE HAM never sees an idle window."""
    nc = tc.nc
    wp = pools["warm"].tile([64, 2], F32, tag="warm", name="warm")
    nc.tensor.matmul(wp[:], lhsT=src[:, :64], rhs=src[:, :2])


def _squash(tc, pools, s_ps, scale):
    """v = t*|t|/(1+t^2) with t = scale*s, all on DVE (no scalar act tables).

    s_ps columns are (o,c); v is written strided so its memory layout is
    (c,o).  Returns SBUF tile [NB, CO] fp32."""
    nc = tc.nc
    sb = pools["sb"]
    t = sb.tile([NB, CO], F32, tag="sq_t")
    sq = sb.tile([NB, CO], F32, tag="sq_sq")
    at = sb.tile([NB, CO], F32, tag="sq_at")
    num = sb.tile([NB, CO], F32, tag="sq_num")
    rv = sb.tile([NB, CO], F32, tag="sq_rv")
    v = sb.tile([NB, CO], F32, tag="sq_v", bufs=2)
    s_ap = s_ps[:, :CO]
    nc.vector.tensor_scalar_mul(t[:], s_ap, scale)             # t
    nc.vector.tensor_mul(sq[:], t[:], t[:])                    # t^2
    I32 = mybir.dt.int32
    nc.vector.tensor_scalar(                                   # |t| (clear sign bit)
        at[:].bitcast(I32), t[:].bitcast(I32), 0x7FFFFFFF, None, ALU.bitwise_and
    )
    _warm_pe(tc, pools, sq)
    nc.vector.tensor_scalar_add(sq[:], sq[:], 1.0)             # 1+t^2
    nc.vector.reciprocal(rv[:], sq[:])
    nc.vector.tensor_mul(num[:], t[:], at[:])                  # t*|t|
    _warm_pe(tc, pools, num)
    # strided write: (o,c) compute order -> (c,o) memory layout
    nc.vector.tensor_mul(
        v[:].rearrange("b (c o) -> b o c", c=C),
        num[:].rearrange("b (o c) -> b o c", o=O),
        rv[:].rearrange("b (o c) -> b o c", o=O),
    )
    _warm_pe(tc, pools, v)
    return v


def _sj_matmuls(tc, pools, xtv, rhsv):
    """s[b, (o,c)] = sum over all 72 chunks: xt_chunk^T @ rhs_chunk (bf16)."""
    nc = tc.nc
    s_ps = pools["pbig"].tile([NB, CO], F32, tag="gbig")
    for k in range(NCHUNK):
        nc.tensor.matmul(
            s_ps[:],
            lhsT=xtv(k),
            rhs=rhsv(k),
            start=(k == 0),
            stop=(k == NCHUNK - 1),
        )
    return s_ps


def _ab_round(tc, pools, xr_t, w4s, v, bones, tag):
    """Full-batch route-sharded b-update round.

    AllGather v (bf16) -> G over all 256 batches for our 9 local chunks ->
    P/o/i reductions -> AllGather the [10,144] slice -> b-update [10,1152]."""
    nc = tc.nc
    pbig, psm, sb, dram = pools["pbig"], pools["psm"], pools["sb"], pools["dram"]

    vb = sb.tile([NB, CO], BF16, tag="vb", bufs=2, name=f"vb{tag}")
    nc.vector.tensor_scalar_mul(vb[:], v[:], 1.0)
    ccv_in = dram.tile([NB, CO], BF16, tag="ccvin", bufs=2, name=f"ccvin{tag}")
    ccv_out = dram.tile(
        [B, CO], BF16, tag="ccvout", addr_space="Shared", bufs=2, name=f"ccvout{tag}"
    )
    nc.sync.dma_start(ccv_in[:], vb[:])
    nc.gpsimd.collective_compute(
        "AllGather",
        ALU.bypass,
        replica_groups=[list(range(NCORES))],
        ins=[ccv_in[:].opt()],
        outs=[ccv_out[:].opt()],
    )
    vh = []
    for h in range(2):
        vt = sb.tile([128, CO], BF16, tag=f"vh{h}", bufs=2, name=f"vh{h}")
        nc.sync.dma_start(vt[:], ccv_out[128 * h : 128 * h + 128, :])
        vh.append(vt)

    # G for 9 local chunks (full batch, K=256 as two accumulating K=128 matmuls)
    pr = sb.tile([128, NLC * C], F32, tag="pr", bufs=2, name=f"pr{tag}")
    for grp, (c0, nch) in enumerate(((0, 6), (6, 3))):
        g_ps = pbig.tile([128, 2 * PSB], F32, tag="gbig")
        for j in range(nch):
            off = PSB * (j // 3) + CO * (j % 3)
            for h in range(2):
                nc.tensor.matmul(
                    g_ps[:, off : off + CO],
                    lhsT=xr_t[h][:, 128 * (c0 + j) : 128 * (c0 + j) + 128],
                    rhs=vh[h][:],
                    start=(h == 0),
                    stop=(h == 1),
                )
        # P = (G/B) .* W4slice
        p_t = sb.tile([128, nch * CO], F32, tag="p", bufs=2)
        na = nch // 3
        g_view = g_ps[:].rearrange("p (a x) -> p a x", a=2)[:, :na, : 3 * CO].rearrange(
            "p a (s e) -> p a s e", s=3
        )
        w_view = w4s[:, CO * c0 : CO * (c0 + nch)].rearrange(
            "p (a s e) -> p a s e", a=na, s=3
        )
        p_view = p_t[:].rearrange("p (a s e) -> p a s e", a=na, s=3)
        nc.vector.scalar_tensor_tensor(
            p_view, g_view, 1.0 / B, w_view, ALU.mult, ALU.mult
        )
        # o-reduce into pr, c-major: free idx 9c + (c0+j)
        pr_view = (
            pr[:]
            .rearrange("p (c a) -> p c a", c=C)[:, :, c0 : c0 + nch]
            .transpose([0, 2, 1])
        )
        nc.vector.tensor_reduce(
            pr_view,
            p_t[:].rearrange("p (a o) -> p a o", o=O),
            axis=AX.X,
            op=ALU.add,
        )
    # i-reduce + transpose all 9 chunks in one matmul: out[9c+a, n] = ab[16a+n, c]
    q_ps = psm.tile([NLC * C, 16], F32, tag="sm", name="q_ps")
    nc.tensor.matmul(q_ps[:], lhsT=pr[:], rhs=bones[:])
    q_sb = sb.tile([NLC * C, 16], F32, tag="q_sb", bufs=2, name="q_sb")
    nc.vector.tensor_scalar_mul(q_sb[:], q_ps[:], 1.0)

    ccab_in = dram.tile([C, RL], F32, tag="ccabin", bufs=2, name=f"ccabin{tag}")
    ccab_out = dram.tile(
        [NCORES * C, RL], F32, tag="ccabout", addr_space="Shared", bufs=2,
        name=f"ccabout{tag}",
    )
    dst = ccab_in[:].rearrange("c (a n) -> c a n", a=NLC)
    nc.sync.dma_start(dst, q_sb[:])
    nc.gpsimd.collective_compute(
        "AllGather",
        ALU.bypass,
        replica_groups=[list(range(NCORES))],
        ins=[ccab_in[:].opt()],
        outs=[ccab_out[:].opt()],
    )
    # gather back as [10, 1152]: b[c, 144*rho + ri] = out[10*rho + c, ri]
    ar = sb.tile([C, R], F32, tag="ar", bufs=2, name=f"ar{tag}")
    src = ccab_out[:].rearrange("(rho c) ri -> c rho ri", c=C)
    nc.sync.dma_start(ar[:].rearrange("c (rho ri) -> c rho ri", rho=NCORES), src)
    return ar


def _softmax_cb_wc(tc, pools, bT, w4v, ident, selrepb, wc_t):
    """c = softmax(b) over routes; build Wc tiles = W4 .* broadcast(c)."""
    nc = tc.nc
    sb, psm = pools["sb"], pools["psm"]
    NRB = R // 128  # 9 route-blocks
    e = sb.tile([C, R], BF16, tag="smx_e")
    ssum = sb.tile([C, 1], F32, tag="smx_s")
    sinv = sb.tile([C, 1], F32, tag="smx_si")
    nc.scalar.activation(e[:], bT[:], ACTF.Exp, accum_out=ssum[:])
    nc.vector.reciprocal(sinv[:], ssum[:])
    nc.vector.tensor_scalar_mul(e[:], e[:], sinv[:])  # e becomes c^T [10, 1152]
    # transpose c to route-major [128, 9*10]: col rb*10+c holds c[128rb+q, c]
    cr_all = sb.tile([128, NRB * C], BF16, tag="cr_all", name="cr_all")
    for rb in range(NRB):
        tp = psm.tile([128, C], BF16, tag="sm", name="tp")
        nc.tensor.transpose(tp[:], e[:, 128 * rb : 128 * rb + 128], ident[:C, :C])
        nc.vector.tensor_scalar_mul(cr_all[:, C * rb : C * rb + C], tp[:], 1.0)
    # replicate over i: for each s in 0..8, one matmul gives cb for chunks k=8rb+s:
    # out[p,(rb,c)] = cr_all[16s + p//8, (rb,c)].  Stored so col k*10+c = chunk k.
    cb_all = sb.tile([128, NCHUNK * C], BF16, tag="cb_all", name="cb_all")
    cb_v = cb_all[:].rearrange("p (rb s c) -> p rb s c", s=8, c=C)
    for s in range(8):
        cb_ps = psm.tile([128, NRB * C], F32, tag="sm", name="cb_ps")
        nc.tensor.matmul(cb_ps[:], lhsT=selrepb[:, 128 * s : 128 * s + 128], rhs=cr_all[:])
        nc.vector.tensor_scalar_mul(
            cb_v[:, :, s, :], cb_ps[:].rearrange("p (rb c) -> p rb c", c=C), 1.0
        )
    # per group: broadcast over o (middle stride-0 dim; last dim c stays packed
    # -> DVE 2x mode), multiply into Wc
    for g in range(NGRP):
        cb_view = (
            cb_all[:, GC * C * g : GC * C * (g + 1)]
            .rearrange("p (j c) -> p j c", c=C)
            .unsqueeze(2)
            .broadcast_to([128, GC, O, C])
        )
        w_view = w4v(g).rearrange("p (j o c) -> p j o c", j=GC, o=O)
        wc_view = wc_t[g][:].rearrange("p (j o c) -> p j o c", j=GC, o=O)
        nc.vector.tensor_mul(wc_view, w_view, cb_view)


def build():
    """Build the Bass module (one program, SPMD across 8 cores)."""
    nc = bacc.Bacc("TRN2", target_bir_lowering=False, debug=False, num_devices=NCORES)

    # chunk-major host layouts: free idx = 32k+b (xt), 160k+oc (w4, (o,c) order)
    d_xt = nc.dram_tensor("xt", [128, NCHUNK * NB], BF16, kind="ExternalInput").ap()
    d_xr = nc.dram_tensor("xr", [B, RL * I], BF16, kind="ExternalInput").ap()
    d_w4 = nc.dram_tensor("w4", [128, NCHUNK * CO], BF16, kind="ExternalInput").ap()
    d_w4s = nc.dram_tensor("w4s", [128, NLC * CO], BF16, kind="ExternalInput").ap()
    d_id = nc.dram_tensor("ident", [128, 128], BF16, kind="ExternalInput").ap()
    d_sr = nc.dram_tensor("selrep", [128, 8 * 128], BF16, kind="ExternalInput").ap()
    d_bo = nc.dram_tensor("bones", [128, 16], F32, kind="ExternalInput").ap()
    d_out = nc.dram_tensor("vout", [NB, CO], F32, kind="ExternalOutput").ap()

    with tile.TileContext(nc) as tc:
        with (
            tc.tile_pool(name="const", bufs=1) as const,
            tc.tile_pool(name="w4p", bufs=1) as w4p,
            tc.tile_pool(name="xtp", bufs=1) as xtp,
            tc.tile_pool(name="wcp", bufs=1) as wcp,
            tc.tile_pool(name="sb", bufs=1) as sb,
            tc.tile_pool(name="pbig", bufs=2, space="PSUM") as pbig,
            tc.tile_pool(name="psm", bufs=2, space="PSUM") as psm,
            tc.tile_pool(name="warm", bufs=1, space="PSUM") as warm,
            tc.tile_pool(name="dram", bufs=1, space="DRAM") as dram,
        ):
            pools = {"sb": sb, "pbig": pbig, "psm": psm, "warm": warm, "dram": dram}

            # two warmup collectives on the otherwise-idle gpsimd queue, issued
            # FIRST so they complete during the input-DMA stream: one tiny (to
            # absorb launch skew) and one shaped like the real v-AllGather (to
            # warm that mesh algorithm path).
            cw_in = dram.tile([1, 8], F32, tag="cwin", name="cw_in")
            cw_out = dram.tile(
                [8, 8], F32, tag="cwout", addr_space="Shared", name="cw_out"
            )
            cw_sb = sb.tile([1, 8], F32, tag="cw_sb")
            nc.gpsimd.memset(cw_sb[:], 0.0)
            nc.gpsimd.dma_start(cw_in[:], cw_sb[:])
            nc.gpsimd.collective_compute(
                "AllGather",
                ALU.bypass,
                replica_groups=[list(range(NCORES))],
                ins=[cw_in[:].opt()],
                outs=[cw_out[:].opt()],
            )
            cw2_in = dram.tile([NB, CO], BF16, tag="cw2in", name="cw2_in")
            cw2_out = dram.tile(
                [B, CO], BF16, tag="cw2out", addr_space="Shared", name="cw2_out"
            )
            cw2_sb = sb.tile([NB, CO], BF16, tag="cw2_sb")
            nc.gpsimd.memset(cw2_sb[:], 0.0)
            nc.gpsimd.dma_start(cw2_in[:], cw2_sb[:])
            nc.gpsimd.collective_compute(
                "AllGather",
                ALU.bypass,
                replica_groups=[list(range(NCORES))],
                ins=[cw2_in[:].opt()],
                outs=[cw2_out[:].opt()],
            )

            # mega-tiles: WSPLIT groups each; first holds group 0 alone so its
            # DMA lands fast and s_j starts while the rest streams.  All input
            # streams ride the scalar HWDGE ring; the sync ring stays free for
            # the latency-critical collective bounce DMAs.
            w4_mt, xt_mt = [], []
            goff = 0
            for ti, ngr in enumerate(WSPLIT):
                wt = w4p.tile([128, ngr * GW], BF16, tag=f"w4m{ti}", name=f"w4m{ti}")
                nc.scalar.dma_start(wt[:], d_w4[:, GW * goff : GW * (goff + ngr)])
                xt_ = xtp.tile(
                    [128, ngr * GC * NB], BF16, tag=f"xtm{ti}", name=f"xtm{ti}"
                )
                nc.scalar.dma_start(
                    xt_[:], d_xt[:, GC * NB * goff : GC * NB * (goff + ngr)]
                )
                w4_mt.append((goff, wt))
                xt_mt.append((goff, xt_))
                goff += ngr

            def w4v(g):
                for off, wt in reversed(w4_mt):
                    if g >= off:
                        return wt[:, GW * (g - off) : GW * (g - off + 1)]

            def xtv(k):
                g, j = k // GC, k % GC
                for off, xt_ in reversed(xt_mt):
                    if g >= off:
                        return xt_[:, NB * (GC * (g - off) + j) : NB * (GC * (g - off) + j) + NB]

            wc_t = []
            for g in range(NGRP):
                wc_t.append(wcp.tile([128, GW], BF16, tag=f"wc_{g}", name=f"wc_{g}"))

            # route-shard inputs (needed from the first a-phase on)
            xr_t = []
            for h in range(2):
                xh = sb.tile([128, RL * I], BF16, tag=f"xr{h}", name=f"xr{h}")
                nc.scalar.dma_start(xh[:], d_xr[128 * h : 128 * h + 128, :])
                xr_t.append(xh)
            w4s = sb.tile([128, NLC * CO], BF16, tag="w4s")
            nc.scalar.dma_start(w4s[:], d_w4s[:])
            ident = const.tile([128, 128], BF16, tag="ident")
            bones = const.tile([128, 16], F32, tag="bones")
            selrepb = const.tile([128, 8 * 128], BF16, tag="selrepb")
            nc.scalar.dma_start(bones[:], d_bo[:])
            nc.scalar.dma_start(ident[:], d_id[:])
            nc.scalar.dma_start(selrepb[:], d_sr[:])

            # ---- iter 0: c uniform -> s = (X @ W4) / R
            s0 = _sj_matmuls(
                tc, pools, xtv,
                lambda k: w4v(k // GC)[:, CO * (k % GC) : CO * (k % GC) + CO],
            )
            v0 = _squash(tc, pools, s0, 1.0 / R)
            bT1 = _ab_round(tc, pools, xr_t, w4s, v0, bones, tag=0)  # b was 0

            # ---- iter 1
            _softmax_cb_wc(tc, pools, bT1, w4v, ident, selrepb, wc_t)
            s1 = _sj_matmuls(
                tc, pools, xtv,
                lambda k: wc_t[k // GC][:, CO * (k % GC) : CO * (k % GC) + CO],
            )
            v1 = _squash(tc, pools, s1, 1.0)
            ar1 = _ab_round(tc, pools, xr_t, w4s, v1, bones, tag=1)
            bT2 = sb.tile([C, R], F32, tag="bT2")
            nc.vector.tensor_add(bT2[:], bT1[:], ar1[:])

            # ---- iter 2 (final)
            _softmax_cb_wc(tc, pools, bT2, w4v, ident, selrepb, wc_t)
            s2 = _sj_matmuls(
                tc, pools, xtv,
                lambda k: wc_t[k // GC][:, CO * (k % GC) : CO * (k % GC) + CO],
            )
            v2 = _squash(tc, pools, s2, 1.0)
            nc.sync.dma_start(d_out[:], v2[:])

    nc.compile()
    return nc


def _host_inputs(x, W):
    """Per-core input maps with pre-arranged layouts."""
    import ml_dtypes

    bf16 = ml_dtypes.bfloat16
    x = np.asarray(x, dtype=np.float32)
    W = np.asarray(W, dtype=np.float32)
    # W4[(r,i), (o,c)] chunk-major in the free dim: [128, 72*160]  (s_j moving)
    wf_oc = np.ascontiguousarray(W.transpose(0, 3, 2, 1)).reshape(RI, CO)
    wfc_oc = wf_oc.reshape(NCHUNK, 128, CO)
    w4h = np.ascontiguousarray(wfc_oc.transpose(1, 0, 2)).reshape(
        128, NCHUNK * CO
    ).astype(bf16)
    # W4[(r,i), (c,o)] local chunks (P-phase, matches v's (c,o) layout)
    wf_co = np.ascontiguousarray(W.transpose(0, 3, 1, 2)).reshape(RI, CO)
    wfc_co = wf_co.reshape(NCHUNK, 128, CO)
    ident = np.eye(128, dtype=bf16)
    # selrep[s][q, p] = 1 iff q == 16s + p//8, packed s-major in free dim
    selrep = np.zeros((8, 128, 128), dtype=np.float32)
    ss, qq, pp = np.meshgrid(np.arange(8), np.arange(128), np.arange(128), indexing="ij")
    selrep[qq == 16 * ss + pp // 8] = 1.0
    selrepb = np.ascontiguousarray(selrep.transpose(1, 0, 2)).reshape(
        128, 8 * 128
    ).astype(bf16)
    pq, jq = np.meshgrid(np.arange(128), np.arange(16), indexing="ij")
    bones = (pq // 8 == jq).astype(np.float32)
    in_maps = []
    for c in range(NCORES):
        xc = np.ascontiguousarray(x[NB * c : NB * (c + 1)].reshape(NB, RI))
        xt = np.ascontiguousarray(xc.T)  # [9216, 32]
        xth = np.ascontiguousarray(
            xt.reshape(NCHUNK, 128, NB).transpose(1, 0, 2)
        ).reshape(128, NCHUNK * NB).astype(bf16)
        xrh = np.ascontiguousarray(x[:, RL * c : RL * (c + 1), :]).reshape(
            B, RL * I
        ).astype(bf16)
        w4sh = np.ascontiguousarray(
            wfc_co[NLC * c : NLC * (c + 1)].transpose(1, 0, 2)
        ).reshape(128, NLC * CO).astype(bf16)
        in_maps.append(
            {
                "xt": xth,
                "xr": xrh,
                "w4": w4h,
                "w4s": w4sh,
                "ident": ident,
                "selrep": selrepb,
                "bones": bones,
            }
        )
    return in_maps


def kernel(x, W, trace=False):
    global _BUILT
    if _BUILT is None:
        _BUILT = build()
    nc = _BUILT
    in_maps = _host_inputs(x, W)
    res = bass_utils.run_bass_kernel_spmd(
        nc, in_maps, core_ids=list(range(NCORES)), trace=trace
    )
    v = np.concatenate([res.results[c]["vout"] for c in range(NCORES)], axis=0)
    out = v.reshape(B, C, O, 1)
    if trace:
        kernel.last_exec_time_ns = res.exec_time_ns
        kernel.last_results = res
    return out


# revision 7
# speedup vs baseline: 1.2594x; 1.0191x over previous
"""DigitCaps dynamic-routing kernel for 8 Trainium2 NeuronCores.

Structure (v2 — route-sharded s_j, one critical-path collective per round):
  - s_j is sharded over ROUTES: each core contracts only its 9 local chunks
    (144 routes x 8 in-ch = 9x128 partitions) for the FULL 256-batch, then an
    AllReduce(add) of s [256,160] completes the route sum.  Every core then
    holds s (and v) for ALL batches, so the a/b-phase needs no v-AllGather.
  - Wc is built UNNORMALIZED (exp(b), no softmax denominator).  The per-capsule
    1/Z lands as a [128,160] broadcast tile (PE K=1 matmul) multiplied in at
    squash time; Z comes from the full b maintained via a small ab-AllGather
    [10,144]->[80,144] that flies in the shadow of the s-AllReduce.
  - Critical-path collectives: 3x AllReduce(s).  Shadowed: 2x AllGather(ab).
    The first collective also absorbs the ~50-60us one-time CC/ncfw cold
    setup + inter-core launch skew, so pre-barrier work is effectively free.

Algebra (never materialize u_hat):
  s_j[b,co]   = sum_{(r,i)} xT[(r,i),b] * (e[r,c]*W4[(r,i),co]),  e = exp(b)
  v           = squash(s_j / Z_c),  Z_c = sum_r e[r,c]
  ab[r,c]     = (1/B) sum_{i,o} W4[(r,i),co] * G[(r,i),co]
  G           = sum_{all b} x[b,(r,i)] v[b,co]   (K=256 as two K=128 matmuls)
  then a DVE o-reduction and one PE matmul vs block-ones that i-reduces AND
  transposes into the ab bounce buffer.

Precision: all big matmuls in bf16 (1 cycle/row); PSUM, collectives, routing
state in fp32.  End-to-end rel err ~4e-3 vs the 2e-2 gate.

Layouts: w4l/wc hold each chunk's 160 cols in (o,c) order so the Wc broadcast
keeps the last AP dim packed (DVE 2x); squash writes v strided into (c,o), so
the ab-phase (G, P, w4s) stays (c,o).  Output is the full [256,160] v; the
host slices each core's own 32 rows.

Engine budget: Scalar runs ONLY Exp; squash + copies run on DVE; scalar HWDGE
ring carries input streams, sync ring the collective bounce DMAs.
"""

import sys
import numpy as np

sys.path.insert(0, "/opt/trn_rl_repo")

import concourse.bass as bass
import concourse.bacc as bacc
import concourse.mybir as mybir
import concourse.tile as tile
from concourse import bass_utils

F32 = mybir.dt.float32
BF16 = mybir.dt.bfloat16
I32 = mybir.dt.int32
ALU = mybir.AluOpType
ACTF = mybir.ActivationFunctionType
AX = mybir.AxisListType

B, R, C, O, I = 256, 1152, 10, 16, 8
NCORES = 8
NB = B // NCORES            # 32 batch rows owned per core (output shard)
RI = R * I                  # 9216 global contraction dim
CO = C * O                  # 160 output cols
NLC = 9                     # local chunks per core (route shard)
RL = R // NCORES            # 144 local routes
PSB = 512                   # PSUM bank size in f32

_BUILT = None


def _warm_pe(tc, pools, src):
    """Tiny dummy matmul keyed on `src` so the PE HAM never sees an idle window."""
    nc = tc.nc
    wp = pools["warm"].tile([64, 2], F32, tag="warm", name="warm")
    nc.tensor.matmul(wp[:], lhsT=src[:, :64], rhs=src[:, :2])


def _s_round(tc, pools, xtl, rhs_fn, tag):
    """Local-route partial s for the full batch + AllReduce.

    Returns [sh0, sh1] SBUF tiles [128, CO] f32 holding the reduced s."""
    nc = tc.nc
    sb, dram = pools["sb"], pools["dram"]
    s_ps = pools["pbig"].tile([128, 2 * PSB], F32, tag="gbig")
    for j in range(NLC):
        for h in range(2):
            nc.tensor.matmul(
                s_ps[:, PSB * h : PSB * h + CO],
                lhsT=xtl[:, B * j + 128 * h : B * j + 128 * h + 128],
                rhs=rhs_fn(j),
                start=(j == 0),
                stop=(j == NLC - 1),
            )
    ccs_in = dram.tile([B, CO], F32, tag="ccsin", bufs=2, name=f"ccsin{tag}")
    ccs_out = dram.tile(
        [B, CO], F32, tag="ccsout", addr_space="Shared", bufs=2, name=f"ccsout{tag}"
    )
    for h in range(2):
        sp = sb.tile([128, CO], F32, tag=f"sp{h}", bufs=2, name=f"sp{h}_{tag}")
        nc.vector.tensor_scalar_mul(sp[:], s_ps[:, PSB * h : PSB * h + CO], 1.0)
        nc.sync.dma_start(ccs_in[128 * h : 128 * h + 128, :], sp[:])
    nc.gpsimd.collective_compute(
        "AllReduce",
        ALU.add,
        replica_groups=[list(range(NCORES))],
        ins=[ccs_in[:].opt()],
        outs=[ccs_out[:].opt()],
    )
    sh = []
    for h in range(2):
        t = sb.tile([128, CO], F32, tag=f"sh{h}", bufs=2, name=f"sh{h}_{tag}")
        nc.sync.dma_start(t[:], ccs_out[128 * h : 128 * h + 128, :])
        sh.append(t)
    return sh


def _squash(tc, pools, sh, scale, zb, tag):
    """v = t*|t|/(1+t^2), t = s*scale or s*zb (broadcast 1/Z), on DVE.

    sh columns are (o,c); v is written strided -> (c,o) memory layout.
    Returns ([vf0,vf1] f32, [vb0,vb1] bf16)."""
    nc = tc.nc
    sb = pools["sb"]
    vf, vb = [], []
    for h in range(2):
        t = sb.tile([128, CO], F32, tag=f"sq_t{h}")
        sq = sb.tile([128, CO], F32, tag=f"sq_sq{h}")
        at = sb.tile([128, CO], F32, tag=f"sq_at{h}")
        num = sb.tile([128, CO], F32, tag=f"sq_num{h}")
        rv = sb.tile([128, CO], F32, tag=f"sq_rv{h}")
        v = sb.tile([128, CO], F32, tag=f"sq_v{h}", bufs=2, name=f"vf{h}_{tag}")
        vbh = sb.tile([128, CO], BF16, tag=f"sq_vb{h}", bufs=2, name=f"vb{h}_{tag}")
        if zb is None:
            nc.vector.tensor_scalar_mul(t[:], sh[h][:], scale)
        else:
            nc.vector.tensor_mul(t[:], sh[h][:], zb[:])
        nc.vector.tensor_mul(sq[:], t[:], t[:])
        nc.vector.tensor_scalar(                       # |t| (clear sign bit)
            at[:].bitcast(I32), t[:].bitcast(I32), 0x7FFFFFFF, None, ALU.bitwise_and
        )
        _warm_pe(tc, pools, sq)
        nc.vector.tensor_scalar_add(sq[:], sq[:], 1.0)
        nc.vector.reciprocal(rv[:], sq[:])
        nc.vector.tensor_mul(num[:], t[:], at[:])
        _warm_pe(tc, pools, num)
        # strided write: (o,c) compute order -> (c,o) memory layout
        nc.vector.tensor_mul(
            v[:].rearrange("b (c o) -> b o c", c=C),
            num[:].rearrange("b (o c) -> b o c", o=O),
            rv[:].rearrange("b (o c) -> b o c", o=O),
        )
        nc.vector.tensor_scalar_mul(vbh[:], v[:], 1.0)  # bf16 cast for G matmuls
        vf.append(v)
        vb.append(vbh)
    return vf, vb


def _ab_phase(tc, pools, xr_t, w4s, vb, bones, tag):
    """G/P/o-i-reductions over the 9 local chunks (all local; v already full).

    Returns (abl [10,144] local b-increment tile, ccab_out for later gather)."""
    nc = tc.nc
    pbig, psm, sb, dram = pools["pbig"], pools["psm"], pools["sb"], pools["dram"]
    pr = sb.tile([128, NLC * C], F32, tag="pr", bufs=2, name=f"pr{tag}")
    for grp, (c0, nch) in enumerate(((0, 6), (6, 3))):
        g_ps = pbig.tile([128, 2 * PSB], F32, tag="gbig")
        for j in range(nch):
            off = PSB * (j // 3) + CO * (j % 3)
            for h in range(2):
                nc.tensor.matmul(
                    g_ps[:, off : off + CO],
                    lhsT=xr_t[h][:, 128 * (c0 + j) : 128 * (c0 + j) + 128],
                    rhs=vb[h][:],
                    start=(h == 0),
                    stop=(h == 1),
                )
        # P = (G/B) .* W4slice
        p_t = sb.tile([128, nch * CO], F32, tag="p", bufs=2)
        na = nch // 3
        g_view = g_ps[:].rearrange("p (a x) -> p a x", a=2)[:, :na, : 3 * CO].rearrange(
            "p a (s e) -> p a s e", s=3
        )
        w_view = w4s[:, CO * c0 : CO * (c0 + nch)].rearrange(
            "p (a s e) -> p a s e", a=na, s=3
        )
        p_view = p_t[:].rearrange("p (a s e) -> p a s e", a=na, s=3)
        nc.vector.scalar_tensor_tensor(
            p_view, g_view, 1.0 / B, w_view, ALU.mult, ALU.mult
        )
        # o-reduce into pr, c-major: free idx 9c + (c0+j)
        pr_view = (
            pr[:]
            .rearrange("p (c a) -> p c a", c=C)[:, :, c0 : c0 + nch]
            .transpose([0, 2, 1])
        )
        nc.vector.tensor_reduce(
            pr_view,
            p_t[:].rearrange("p (a o) -> p a o", o=O),
            axis=AX.X,
            op=ALU.add,
        )
    # i-reduce + transpose all 9 chunks in one matmul: out[9c+a, n] = ab[16a+n, c]
    q_ps = psm.tile([NLC * C, 16], F32, tag="sm", name="q_ps")
    nc.tensor.matmul(q_ps[:], lhsT=pr[:], rhs=bones[:])
    q_sb = sb.tile([NLC * C, 16], F32, tag="q_sb", bufs=2, name="q_sb")
    nc.vector.tensor_scalar_mul(q_sb[:], q_ps[:], 1.0)

    # local b increment [10, 144] (scalar ring; input stream is long done)
    abl = sb.tile([C, RL], F32, tag="abl", bufs=2, name=f"abl{tag}")
    nc.scalar.dma_start(abl[:].rearrange("c (a n) -> c a n", a=NLC), q_sb[:])

    # shadow AllGather of the ab slice (full-b for Z only)
    ccab_in = dram.tile([C, RL], F32, tag="ccabin", bufs=2, name=f"ccabin{tag}")
    ccab_out = dram.tile(
        [NCORES * C, RL], F32, tag="ccabout", addr_space="Shared", bufs=2,
        name=f"ccabout{tag}",
    )
    nc.sync.dma_start(ccab_in[:].rearrange("c (a n) -> c a n", a=NLC), q_sb[:])
    nc.gpsimd.collective_compute(
        "AllGather",
        ALU.bypass,
        replica_groups=[list(range(NCORES))],
        ins=[ccab_in[:].opt()],
        outs=[ccab_out[:].opt()],
    )
    return abl, ccab_out


def _gather_ar(tc, pools, ccab_out, tag):
    """Gather the ab AllGather into [10, 1152] full-route order."""
    nc = tc.nc
    ar = pools["sb"].tile([C, R], F32, tag="ar", bufs=2, name=f"ar{tag}")
    src = ccab_out[:].rearrange("(rho c) ri -> c rho ri", c=C)
    nc.scalar.dma_start(ar[:].rearrange("c (rho ri) -> c rho ri", rho=NCORES), src)
    return ar


def _cb_wc(tc, pools, b_loc, w4l, identm, sel128, sel16, wc_t, tag):
    """e=exp(b_loc) (unnormalized), expand to per-(r,i) cb, Wc = W4l .* cb."""
    nc = tc.nc
    sb, psm = pools["sb"], pools["psm"]
    e = sb.tile([C, RL], BF16, tag="smx_e")
    nc.scalar.activation(e[:], b_loc[:], ACTF.Exp)
    # transpose local c to route-major: t0 for routes 0..127, t1 for 128..143
    t0_ps = psm.tile([128, C], BF16, tag="sm", name="t0_ps")
    nc.tensor.transpose(t0_ps[:], e[:, :128], identm[:C, :C])
    t0 = sb.tile([128, C], BF16, tag="t0", name=f"t0_{tag}")
    nc.vector.tensor_scalar_mul(t0[:], t0_ps[:], 1.0)
    t1_ps = psm.tile([16, C], BF16, tag="sm", name="t1_ps")
    nc.tensor.transpose(t1_ps[:], e[:, 128:144], identm[:C, :C])
    t1 = sb.tile([16, C], BF16, tag="t1", name=f"t1_{tag}")
    nc.vector.tensor_scalar_mul(t1[:], t1_ps[:], 1.0)
    # replicate over i: chunk j<=7 from t0 via sel128 block j; chunk 8 from t1
    cb_all = sb.tile([128, NLC * C], BF16, tag="cb_all", name=f"cb_all{tag}")
    for s in range(8):
        cb_ps = psm.tile([128, C], F32, tag="sm", name="cb_ps")
        nc.tensor.matmul(cb_ps[:], lhsT=sel128[:, 128 * s : 128 * s + 128], rhs=t0[:])
        nc.vector.tensor_scalar_mul(cb_all[:, C * s : C * s + C], cb_ps[:], 1.0)
    cb8_ps = psm.tile([128, C], F32, tag="sm", name="cb8_ps")
    nc.tensor.matmul(cb8_ps[:], lhsT=sel16[:16, :], rhs=t1[:])
    nc.vector.tensor_scalar_mul(cb_all[:, C * 8 : C * 9], cb8_ps[:], 1.0)
    # Wc = W4l .* broadcast(cb) — o is a stride-0 middle dim, c stays packed (2x)
    cb_view = (
        cb_all[:]
        .rearrange("p (j c) -> p j c", c=C)
        .unsqueeze(2)
        .broadcast_to([128, NLC, O, C])
    )
    w_view = w4l[:].rearrange("p (j o c) -> p j o c", j=NLC, o=O)
    wc_view = wc_t[:].rearrange("p (j o c) -> p j o c", j=NLC, o=O)
    nc.vector.tensor_mul(wc_view, w_view, cb_view)


def _zb_build(tc, pools, bT, identm, ones1, tag):
    """Z_c = sum_r exp(bT[c,r]); return PSUM tile [128, CO] of 1/Z broadcast
    over partitions and o ((o,c) column order)."""
    nc = tc.nc
    sb, psm = pools["sb"], pools["psm"]
    ef = sb.tile([C, R], BF16, tag="zb_ef")
    zt = sb.tile([C, 1], F32, tag="zb_z")
    nc.scalar.activation(ef[:], bT[:], ACTF.Exp, accum_out=zt[:])
    zrf = sb.tile([C, 1], F32, tag="zb_zrf")
    nc.vector.reciprocal(zrf[:], zt[:])
    zr = sb.tile([C, 1], BF16, tag="zb_zr")
    nc.vector.tensor_scalar_mul(zr[:], zrf[:], 1.0)  # bf16 1/Z
    zrT_ps = psm.tile([1, C], F32, tag="sm", name=f"zrT{tag}")
    nc.tensor.matmul(zrT_ps[:], lhsT=zr[:], rhs=identm[:C, :C])
    zrow = sb.tile([1, CO], BF16, tag="zb_row")
    nc.vector.tensor_scalar_mul(
        zrow[:].rearrange("p (o c) -> p o c", o=O),
        zrT_ps[:].unsqueeze(1).broadcast_to([1, O, C]),
        1.0,
    )
    zb_ps = pools["zb"].tile([128, CO], F32, tag="zb", name=f"zb{tag}")
    nc.tensor.matmul(zb_ps[:], lhsT=ones1[:1, :], rhs=zrow[:])
    return zb_ps


def build():
    """Build the Bass module (one program, SPMD across 8 cores)."""
    nc = bacc.Bacc("TRN2", target_bir_lowering=False, debug=False, num_devices=NCORES)

    # local-chunk-major host layouts; w4l free idx per chunk is (o,c)
    d_xtl = nc.dram_tensor("xtl", [128, NLC * B], BF16, kind="ExternalInput").ap()
    d_xr = nc.dram_tensor("xr", [B, RL * I], BF16, kind="ExternalInput").ap()
    d_w4l = nc.dram_tensor("w4l", [128, NLC * CO], BF16, kind="ExternalInput").ap()
    d_w4s = nc.dram_tensor("w4s", [128, NLC * CO], BF16, kind="ExternalInput").ap()
    d_id = nc.dram_tensor("identm", [16, 16], BF16, kind="ExternalInput").ap()
    d_sr = nc.dram_tensor("sel128", [128, 8 * 128], BF16, kind="ExternalInput").ap()
    d_s16 = nc.dram_tensor("sel16", [16, 128], BF16, kind="ExternalInput").ap()
    d_on = nc.dram_tensor("ones1", [1, 128], BF16, kind="ExternalInput").ap()
    d_bo = nc.dram_tensor("bones", [128, 16], F32, kind="ExternalInput").ap()
    d_out = nc.dram_tensor("vout", [B, CO], F32, kind="ExternalOutput").ap()

    with tile.TileContext(nc) as tc:
        with (
            tc.tile_pool(name="const", bufs=1) as const,
            tc.tile_pool(name="wcp", bufs=2) as wcp,
            tc.tile_pool(name="sb", bufs=1) as sb,
            tc.tile_pool(name="pbig", bufs=2, space="PSUM") as pbig,
            tc.tile_pool(name="psm", bufs=2, space="PSUM") as psm,
            tc.tile_pool(name="zb", bufs=1, space="PSUM") as zbp,
            tc.tile_pool(name="warm", bufs=1, space="PSUM") as warm,
            tc.tile_pool(name="dram", bufs=1, space="DRAM") as dram,
        ):
            pools = {
                "sb": sb, "pbig": pbig, "psm": psm, "zb": zbp, "warm": warm,
                "dram": dram,
            }

            # input streams on the scalar HWDGE ring (sync ring stays free for
            # the latency-critical collective bounce DMAs)
            xtl = sb.tile([128, NLC * B], BF16, tag="xtl", name="xtl")
            nc.scalar.dma_start(xtl[:], d_xtl[:])
            w4l = sb.tile([128, NLC * CO], BF16, tag="w4l", name="w4l")
            nc.scalar.dma_start(w4l[:], d_w4l[:])
            xr_t = []
            for h in range(2):
                xh = sb.tile([128, RL * I], BF16, tag=f"xr{h}", name=f"xr{h}")
                nc.scalar.dma_start(xh[:], d_xr[128 * h : 128 * h + 128, :])
                xr_t.append(xh)
            w4s = sb.tile([128, NLC * CO], BF16, tag="w4s")
            nc.scalar.dma_start(w4s[:], d_w4s[:])
            identm = const.tile([16, 16], BF16, tag="identm")
            sel128 = const.tile([128, 8 * 128], BF16, tag="sel128")
            sel16 = const.tile([16, 128], BF16, tag="sel16")
            ones1 = const.tile([1, 128], BF16, tag="ones1")
            bones = const.tile([128, 16], F32, tag="bones")
            nc.scalar.dma_start(identm[:], d_id[:])
            nc.scalar.dma_start(bones[:], d_bo[:])
            nc.scalar.dma_start(sel16[:], d_s16[:])
            nc.scalar.dma_start(ones1[:], d_on[:])
            nc.scalar.dma_start(sel128[:], d_sr[:])

            # ---- iter 0: c uniform -> s = sum(X W4)/R; 1/R folded into squash
            sh0 = _s_round(
                tc, pools, xtl, lambda j: w4l[:, CO * j : CO * j + CO], tag=0
            )
            vf0, vb0 = _squash(tc, pools, sh0, 1.0 / R, None, tag=0)
            abl0, cco0 = _ab_phase(tc, pools, xr_t, w4s, vb0, bones, tag=0)

            # ---- iter 1 (b1 = ab0)
            wc1 = wcp.tile([128, NLC * CO], BF16, tag="wc", name="wc1")
            _cb_wc(tc, pools, abl0, w4l, identm, sel128, sel16, wc1, tag=1)
            sh1 = _s_round(
                tc, pools, xtl, lambda j: wc1[:, CO * j : CO * j + CO], tag=1
            )
            # in the s-AllReduce shadow: full b1 and its 1/Z broadcast
            bT1 = _gather_ar(tc, pools, cco0, tag=0)
            zb1 = _zb_build(tc, pools, bT1, identm, ones1, tag=1)
            vf1, vb1 = _squash(tc, pools, sh1, 1.0, zb1, tag=1)
            abl1, cco1 = _ab_phase(tc, pools, xr_t, w4s, vb1, bones, tag=1)
            b_loc2 = sb.tile([C, RL], F32, tag="bloc2")
            nc.vector.tensor_add(b_loc2[:], abl0[:], abl1[:])

            # ---- iter 2 (final)
            wc2 = wcp.tile([128, NLC * CO], BF16, tag="wc", name="wc2")
            _cb_wc(tc, pools, b_loc2, w4l, identm, sel128, sel16, wc2, tag=2)
            sh2 = _s_round(
                tc, pools, xtl, lambda j: wc2[:, CO * j : CO * j + CO], tag=2
            )
            ar1 = _gather_ar(tc, pools, cco1, tag=1)
            bT2 = sb.tile([C, R], F32, tag="bT2")
            nc.vector.tensor_add(bT2[:], bT1[:], ar1[:])
            zb2 = _zb_build(tc, pools, bT2, identm, ones1, tag=2)
            vf2, _ = _squash(tc, pools, sh2, 1.0, zb2, tag=2)
            for h in range(2):
                nc.sync.dma_start(d_out[128 * h : 128 * h + 128, :], vf2[h][:])

    nc.compile()
    return nc


def _host_inputs(x, W):
    """Per-core input maps with pre-arranged layouts."""
    import ml_dtypes

    bf16 = ml_dtypes.bfloat16
    x = np.asarray(x, dtype=np.float32)
    W = np.asarray(W, dtype=np.float32)
    # W4[(r,i), (o,c)] rows for the s_j moving operand
    wf_oc = np.ascontiguousarray(W.transpose(0, 3, 2, 1)).reshape(RI, CO)
    wfc_oc = wf_oc.reshape(RI // 128, 128, CO)
    # W4[(r,i), (c,o)] rows for the P-phase (matches v's (c,o) layout)
    wf_co = np.ascontiguousarray(W.transpose(0, 3, 1, 2)).reshape(RI, CO)
    wfc_co = wf_co.reshape(RI // 128, 128, CO)
    identm = np.eye(16, dtype=bf16)
    # sel128[s][q, p] = 1 iff q == 16s + p//8, packed s-major in free dim
    selrep = np.zeros((8, 128, 128), dtype=np.float32)
    ss, qq, pp = np.meshgrid(np.arange(8), np.arange(128), np.arange(128), indexing="ij")
    selrep[qq == 16 * ss + pp // 8] = 1.0
    sel128 = np.ascontiguousarray(selrep.transpose(1, 0, 2)).reshape(
        128, 8 * 128
    ).astype(bf16)
    q16, p16 = np.meshgrid(np.arange(16), np.arange(128), indexing="ij")
    sel16 = (q16 == p16 // 8).astype(bf16)
    ones1 = np.ones((1, 128), dtype=bf16)
    pq, jq = np.meshgrid(np.arange(128), np.arange(16), indexing="ij")
    bones = (pq // 8 == jq).astype(np.float32)
    xf = x.reshape(B, RI)
    in_maps = []
    for c in range(NCORES):
        lo, hi = RL * I * c, RL * I * (c + 1)
        xl = np.ascontiguousarray(xf[:, lo:hi].T)  # [1152, 256] local (r,i) rows
        xtlh = np.ascontiguousarray(
            xl.reshape(NLC, 128, B).transpose(1, 0, 2)
        ).reshape(128, NLC * B).astype(bf16)
        xrh = xf[:, lo:hi].astype(bf16)
        w4lh = np.ascontiguousarray(
            wfc_oc[NLC * c : NLC * (c + 1)].transpose(1, 0, 2)
        ).reshape(128, NLC * CO).astype(bf16)
        w4sh = np.ascontiguousarray(
            wfc_co[NLC * c : NLC * (c + 1)].transpose(1, 0, 2)
        ).reshape(128, NLC * CO).astype(bf16)
        in_maps.append(
            {
                "xtl": xtlh,
                "xr": xrh,
                "w4l": w4lh,
                "w4s": w4sh,
                "identm": identm,
                "sel128": sel128,
                "sel16": sel16,
                "ones1": ones1,
                "bones": bones,
            }
        )
    return in_maps


def kernel(x, W, trace=False):
    global _BUILT
    if _BUILT is None:
        _BUILT = build()
    nc = _BUILT
    in_maps = _host_inputs(x, W)
    res = bass_utils.run_bass_kernel_spmd(
        nc, in_maps, core_ids=list(range(NCORES)), trace=trace
    )
    v = np.concatenate(
        [res.results[c]["vout"][NB * c : NB * (c + 1)] for c in range(NCORES)], axis=0
    )
    out = v.reshape(B, C, O, 1)
    if trace:
        kernel.last_exec_time_ns = res.exec_time_ns
        kernel.last_results = res
    return out


# revision 19
# speedup vs baseline: 1.3089x; 1.0393x over previous
"""DigitCaps dynamic-routing kernel for 8 Trainium2 NeuronCores.

Structure (v3 — hybrid sharding, minimal critical-path collectives):
  - iter 0 (uniform c): BATCH-sharded s0 — each core computes s for its own 32
    batches over all 1152 routes (full W4 streams in pre-barrier, which is
    free: the first collective's ~58us one-time CC/ncfw cold-setup window
    swallows it).  v0 is then AllGather'd ([32,160]->[256,160] bf16), and that
    AllGather also completes inside the cold window.
  - iters 1-2: ROUTE-sharded s_j — each core contracts only its 9 local
    chunks for the full 256-batch with UNNORMALIZED Wc = W4*exp(b); an
    AllReduce(add) of [257,160] bf16 completes the route sum AND carries the
    10 softmax denominators Z_c as row 256.  1/Z is applied at squash time
    via a PE-broadcast [128,160] tile.  No v-AllGather, no ab-AllGather, no
    full-b maintenance: only 2 critical-path collectives after the cold
    window.
  - b state lives in the i-reduce output layout ("q-layout" [90,16], row
    10a+c = local chunk a, capsule c, col n = route 16a+n), so the per-round
    exp/softmax-prep is one Exp + one PE transpose + one select matmul.

Algebra (never materialize u_hat):
  s_j[b,co]   = sum_{(r,i)} xT[(r,i),b] * (e[r,c]*W4[(r,i),co]),  e = exp(b)
  v           = squash(s_j / Z_c),  Z_c = sum_r e[r,c]  (ridden in the AR)
  ab[r,c]     = (1/B) sum_{i,o} W4[(r,i),co] * G[(r,i),co]
  G           = sum_{all b} x[b,(r,i)] v[b,co]   (K=256 as two K=128 matmuls)

Precision: all big matmuls bf16 (1 cycle/row); PSUM f32; collective payloads
bf16.  End-to-end rel err ~5e-3 vs the 2e-2 gate.

Layouts: w4/wc chunks use (o,c) column order so the Wc broadcast keeps the
last AP dim packed (DVE 2x); squash writes v strided into (c,o) so the
ab-phase (G, P, w4s) stays (c,o).  Output is the full [256,160] v; the host
slices each core's own 32 rows.
"""

import sys
import numpy as np

sys.path.insert(0, "/opt/trn_rl_repo")

import concourse.bass as bass
import concourse.bacc as bacc
import concourse.mybir as mybir
import concourse.tile as tile
from concourse import bass_utils

F32 = mybir.dt.float32
BF16 = mybir.dt.bfloat16
I32 = mybir.dt.int32
ALU = mybir.AluOpType
ACTF = mybir.ActivationFunctionType
AX = mybir.AxisListType

B, R, C, O, I = 256, 1152, 10, 16, 8
NCORES = 8
NB = B // NCORES            # 32 batch rows owned per core (output shard)
RI = R * I                  # 9216 global contraction dim
CO = C * O                  # 160 output cols
NCHUNK = RI // 128          # 72 global chunks (iter-0 path)
NGRP = 12                   # chunk groups for the iter-0 w4 mega-tiles
GC = NCHUNK // NGRP         # 6 chunks per group
GW = GC * CO
WSPLIT = (1, 3, 4, 4)       # groups per mega-tile (first alone -> early start)
NLC = 9                     # local chunks per core (route shard)
RL = R // NCORES            # 144 local routes
QP = NLC * C + 6            # q-layout partitions (90) padded to 96
PSB = 512                   # PSUM bank size in f32

_BUILT = None


def _warm_pe(tc, pools, src):
    """Tiny dummy matmul keyed on `src` so the PE HAM never sees an idle window."""
    nc = tc.nc
    wp = pools["warm"].tile([64, 2], F32, tag="warm", name="warm")
    nc.tensor.matmul(wp[:], lhsT=src[:, :64], rhs=src[:, :2])


def _squash_tile(tc, pools, s_ap, np_, scale, zb, vtag, bufs=2):
    """v = t*|t|/(1+t^2), t = s*scale or s.*zb, on DVE; (o,c) in, (c,o) out."""
    nc = tc.nc
    sb = pools["sb"]
    t = sb.tile([np_, CO], F32, tag=f"sq_t{vtag}")
    sq = sb.tile([np_, CO], F32, tag=f"sq_sq{vtag}")
    at = sb.tile([np_, CO], F32, tag=f"sq_at{vtag}")
    num = sb.tile([np_, CO], F32, tag=f"sq_num{vtag}")
    rv = sb.tile([np_, CO], F32, tag=f"sq_rv{vtag}")
    v = sb.tile([np_, CO], F32, tag=f"sq_v{vtag}", bufs=bufs, name=f"v{vtag}")
    if zb is None:
        nc.vector.tensor_scalar_mul(t[:], s_ap, scale)
    else:
        nc.vector.tensor_mul(t[:], s_ap, zb[:np_, :])
    nc.vector.tensor_mul(sq[:], t[:], t[:])
    nc.vector.tensor_scalar(                       # |t| (clear sign bit)
        at[:].bitcast(I32), t[:].bitcast(I32), 0x7FFFFFFF, None, ALU.bitwise_and
    )
    _warm_pe(tc, pools, sq)
    nc.vector.tensor_scalar_add(sq[:], sq[:], 1.0)
    nc.vector.reciprocal_approx_fast(rv[:], sq[:])  # 1+t^2 >= 1: approx safe
    nc.vector.tensor_mul(num[:], t[:], at[:])
    _warm_pe(tc, pools, num)
    nc.vector.tensor_mul(                          # strided: (o,c) -> (c,o)
        v[:].rearrange("b (c o) -> b o c", c=C),
        num[:].rearrange("b (o c) -> b o c", o=O),
        rv[:].rearrange("b (o c) -> b o c", o=O),
    )
    return v


def _ab_q(tc, pools, xr_t, w4s, vh, bones, tag):
    """G/P/o-i-reductions over the 9 local chunks -> q_ps [90,16] PSUM.

    q[10a+c, n] = ab[local route 16a+n, capsule c] (j-major pr layout)."""
    nc = tc.nc
    pbig, psm, sb = pools["pbig"], pools["psm"], pools["sb"]
    pr = sb.tile([128, NLC * C], BF16, tag="pr", bufs=2, name=f"pr{tag}")
    for grp, (c0, nch) in enumerate(((0, 6), (6, 3))):
        g_ps = pbig.tile([128, 2 * PSB], F32, tag="gbig")
        for j in range(nch):
            off = PSB * (j // 3) + CO * (j % 3)
            for h in range(2):
                nc.tensor.matmul(
                    g_ps[:, off : off + CO],
                    lhsT=xr_t[h][:, 128 * (c0 + j) : 128 * (c0 + j) + 128],
                    rhs=vh[h][:],
                    start=(h == 0),
                    stop=(h == 1),
                )
        # P = (G/B) .* W4slice
        p_t = sb.tile([128, nch * CO], BF16, tag="p", bufs=2)
        na = nch // 3
        g_view = g_ps[:].rearrange("p (a x) -> p a x", a=2)[:, :na, : 3 * CO].rearrange(
            "p a (s e) -> p a s e", s=3
        )
        w_view = w4s[:, CO * c0 : CO * (c0 + nch)].rearrange(
            "p (a s e) -> p a s e", a=na, s=3
        )
        p_view = p_t[:].rearrange("p (a s e) -> p a s e", a=na, s=3)
        nc.vector.scalar_tensor_tensor(
            p_view, g_view, 1.0 / B, w_view, ALU.mult, ALU.mult
        )
        # o-reduce into pr, j-major (packed out -> DVE 2x): free idx 10j + c
        with nc.allow_low_precision(reason="bf16 ab partials, 16-term o-sums"):
            nc.vector.tensor_reduce(
                pr[:, C * c0 : C * (c0 + nch)],
                p_t[:].rearrange("p (x o) -> p x o", o=O),
                axis=AX.X,
                op=ALU.add,
            )
    # i-reduce + transpose all 9 chunks in one matmul
    q_ps = psm.tile([NLC * C, 16], F32, tag="sm", name=f"q_ps{tag}")
    nc.tensor.matmul(q_ps[:], lhsT=pr[:], rhs=bones[:])
    return q_ps


def _wc_from_bq(tc, pools, b_q, w4l, identq, sel16, wc_t, zl_b, tag):
    """q-layout softmax prep: e=exp(b_q) (unnormalized), local Z sums into
    zl_b [90->10 via select matmul later], cb via ONE transpose + ONE select
    matmul, Wc = W4l .* broadcast(cb)."""
    nc = tc.nc
    sb, psm = pools["sb"], pools["psm"]
    e_q = sb.tile([NLC * C, 16], BF16, tag="e_q")
    eacc = sb.tile([NLC * C, 1], F32, tag="eacc")
    nc.scalar.activation(e_q[:], b_q[:], ACTF.Exp, accum_out=eacc[:])
    nc.vector.tensor_scalar_mul(zl_b[:], eacc[:], 1.0)  # bf16 cast of row sums
    # eT[n, 10a+c] = e_q[10a+c, n]
    eT_ps = psm.tile([16, NLC * C], BF16, tag="sm", name=f"eT{tag}")
    nc.tensor.transpose(eT_ps[:], e_q[:], identq[: NLC * C, : NLC * C])
    eT = sb.tile([16, NLC * C], BF16, tag="eT", name=f"eTs{tag}")
    nc.vector.tensor_scalar_mul(eT[:], eT_ps[:], 1.0)
    # cb[p, (a,c)] = eT[p//8, (a,c)] — i-replication via one select matmul
    cb_ps = psm.tile([128, NLC * C], F32, tag="sm", name=f"cb{tag}")
    nc.tensor.matmul(cb_ps[:], lhsT=sel16[:16, :], rhs=eT[:])
    cb = sb.tile([128, NLC * C], BF16, tag="cb_all", name=f"cb_all{tag}")
    nc.vector.tensor_scalar_mul(cb[:], cb_ps[:], 1.0)
    # Wc = W4l .* broadcast(cb): o is a stride-0 middle dim, c packed (2x)
    cb_view = (
        cb[:].rearrange("p (j c) -> p j c", c=C).unsqueeze(2).broadcast_to(
            [128, NLC, O, C]
        )
    )
    w_view = w4l[:].rearrange("p (j o c) -> p j o c", j=NLC, o=O)
    wc_view = wc_t[:].rearrange("p (j o c) -> p j o c", j=NLC, o=O)
    nc.vector.tensor_mul(wc_view, w_view, cb_view)


def _s_round(tc, pools, xtl, wc_t, zl_b, sel10, tag):
    """Route-sharded s for the full batch + Z row, AllReduce'd.

    Returns (sh tiles [2][128,CO] bf16, zrow_in [1,C+?] bf16)."""
    nc = tc.nc
    sb, psm, dram = pools["sb"], pools["psm"], pools["dram"]
    s_ps = pools["pbig"].tile([128, 2 * PSB], F32, tag="gbig")
    for j in range(NLC):
        for h in range(2):
            nc.tensor.matmul(
                s_ps[:, PSB * h : PSB * h + CO],
                lhsT=xtl[:, B * j + 128 * h : B * j + 128 * h + 128],
                rhs=wc_t[:, CO * j : CO * j + CO],
                start=(j == 0),
                stop=(j == NLC - 1),
            )
    # local Z_c = sum_{a} zl_b[10a+c]: one tiny select matmul -> [10, 1]
    zl_ps = psm.tile([C, 1], F32, tag="sm", name=f"zl{tag}")
    nc.tensor.matmul(zl_ps[:], lhsT=sel10[: NLC * C, :C], rhs=zl_b[:])
    zlr = sb.tile([C, 1], BF16, tag="zlr")
    nc.vector.tensor_scalar_mul(zlr[:], zl_ps[:], 1.0)
    # transpose to a row [1, 10] for the payload tail
    zrow_ps = psm.tile([1, C], F32, tag="sm", name=f"zrow{tag}")
    nc.tensor.matmul(zrow_ps[:], lhsT=zlr[:], rhs=sel10[:C, :C])  # ident10 block
    zrow_b = sb.tile([1, CO], BF16, tag="zrow_b")
    nc.vector.memset(zrow_b[:], 0.0)
    nc.vector.tensor_scalar_mul(zrow_b[:, :C], zrow_ps[:], 1.0)

    ccs_in = dram.tile([B + 1, CO], BF16, tag="ccsin", bufs=2, name=f"ccsin{tag}")
    ccs_out = dram.tile(
        [B + 1, CO], BF16, tag="ccsout", addr_space="Shared", bufs=2,
        name=f"ccsout{tag}",
    )
    for h in range(2):
        sp = sb.tile([128, CO], BF16, tag=f"sp{h}", bufs=2, name=f"sp{h}_{tag}")
        nc.vector.tensor_scalar_mul(sp[:], s_ps[:, PSB * h : PSB * h + CO], 1.0)
        nc.sync.dma_start(ccs_in[128 * h : 128 * h + 128, :], sp[:])
    nc.sync.dma_start(ccs_in[B : B + 1, :], zrow_b[:])
    nc.gpsimd.collective_compute(
        "AllReduce",
        ALU.add,
        replica_groups=[list(range(NCORES))],
        ins=[ccs_in[:].opt()],
        outs=[ccs_out[:].opt()],
    )
    sh = []
    for h in range(2):
        t = sb.tile([128, CO], BF16, tag=f"sh{h}", bufs=2, name=f"sh{h}_{tag}")
        nc.sync.dma_start(t[:], ccs_out[128 * h : 128 * h + 128, :])
        sh.append(t)
    zin = sb.tile([1, CO], BF16, tag="zin", bufs=2, name=f"zin{tag}")
    nc.sync.dma_start(zin[:], ccs_out[B : B + 1, :])
    return sh, zin


def _zb_build(tc, pools, zin, identm, ones1, tag):
    """1/Z broadcast tile [128, CO] ((o,c) order) from the AR'd Z row."""
    nc = tc.nc
    sb = pools["sb"]
    zf = sb.tile([1, C], F32, tag="zb_zf")
    nc.vector.tensor_scalar_mul(zf[:], zin[:, :C], 1.0)
    zrf = sb.tile([1, C], F32, tag="zb_zrf")
    nc.vector.reciprocal_approx_fast(zrf[:], zf[:])   # Z > 0: approx safe
    zrb = sb.tile([1, C], BF16, tag="zb_zrb")
    nc.vector.tensor_scalar_mul(zrb[:], zrf[:], 1.0)
    zrow = sb.tile([1, CO], BF16, tag="zb_row")
    nc.vector.tensor_scalar_mul(                      # broadcast over o
        zrow[:].rearrange("p (o c) -> p o c", o=O),
        zrb[:].unsqueeze(1).broadcast_to([1, O, C]),
        1.0,
    )
    zb_ps = pools["zb"].tile([128, CO], F32, tag="zb", name=f"zb{tag}")
    nc.tensor.matmul(zb_ps[:], lhsT=ones1[:1, :], rhs=zrow[:])
    return zb_ps


def build():
    """Build the Bass module (one program, SPMD across 8 cores)."""
    nc = bacc.Bacc("TRN2", target_bir_lowering=False, debug=False, num_devices=NCORES)

    d_xt = nc.dram_tensor("xt", [128, NCHUNK * NB], BF16, kind="ExternalInput").ap()
    d_w4 = nc.dram_tensor("w4", [128, NCHUNK * CO], BF16, kind="ExternalInput").ap()
    d_xtl = nc.dram_tensor("xtl", [128, NLC * B], BF16, kind="ExternalInput").ap()
    d_xr = nc.dram_tensor("xr", [B, RL * I], BF16, kind="ExternalInput").ap()
    d_w4l = nc.dram_tensor("w4l", [128, NLC * CO], BF16, kind="ExternalInput").ap()
    d_w4s = nc.dram_tensor("w4s", [128, NLC * CO], BF16, kind="ExternalInput").ap()
    d_idq = nc.dram_tensor("identq", [QP, QP], BF16, kind="ExternalInput").ap()
    d_s16 = nc.dram_tensor("sel16", [16, 128], BF16, kind="ExternalInput").ap()
    d_s10 = nc.dram_tensor("sel10", [QP, 16], BF16, kind="ExternalInput").ap()
    d_on = nc.dram_tensor("ones1", [1, 128], BF16, kind="ExternalInput").ap()
    d_bo = nc.dram_tensor("bones", [128, 16], BF16, kind="ExternalInput").ap()
    d_out = nc.dram_tensor("vout", [B, CO], F32, kind="ExternalOutput").ap()

    with tile.TileContext(nc) as tc:
        with (
            tc.tile_pool(name="const", bufs=1) as const,
            tc.tile_pool(name="w4p", bufs=1) as w4p,
            tc.tile_pool(name="xtp", bufs=1) as xtp,
            tc.tile_pool(name="wcp", bufs=2) as wcp,
            tc.tile_pool(name="sb", bufs=1) as sb,
            tc.tile_pool(name="pbig", bufs=2, space="PSUM") as pbig,
            tc.tile_pool(name="psm", bufs=2, space="PSUM") as psm,
            tc.tile_pool(name="zb", bufs=1, space="PSUM") as zbp,
            tc.tile_pool(name="warm", bufs=1, space="PSUM") as warm,
            tc.tile_pool(name="dram", bufs=1, space="DRAM") as dram,
        ):
            pools = {
                "sb": sb, "pbig": pbig, "psm": psm, "zb": zbp, "warm": warm,
                "dram": dram,
            }

            # minimal warmup collective, triggered at ~t=0, so the ~58us
            # one-time CC/ncfw cold setup starts immediately.
            cw_in = dram.tile([1, 8], F32, tag="cwin", name="cw_in")
            cw_out = dram.tile(
                [8, 8], F32, tag="cwout", addr_space="Shared", name="cw_out"
            )
            cw_sb = sb.tile([1, 8], F32, tag="cw_sb")
            nc.gpsimd.memset(cw_sb[:], 0.0)
            nc.gpsimd.dma_start(cw_in[:], cw_sb[:])
            nc.gpsimd.collective_compute(
                "AllGather",
                ALU.bypass,
                replica_groups=[list(range(NCORES))],
                ins=[cw_in[:].opt()],
                outs=[cw_out[:].opt()],
            )

            # iter-0 input mega-tiles (batch-sharded path), scalar HWDGE ring
            w4_mt, xt_mt = [], []
            goff = 0
            for ti, ngr in enumerate(WSPLIT):
                wt = w4p.tile([128, ngr * GW], BF16, tag=f"w4m{ti}", name=f"w4m{ti}")
                nc.scalar.dma_start(wt[:], d_w4[:, GW * goff : GW * (goff + ngr)])
                xt_ = xtp.tile(
                    [128, ngr * GC * NB], BF16, tag=f"xtm{ti}", name=f"xtm{ti}"
                )
                nc.scalar.dma_start(
                    xt_[:], d_xt[:, GC * NB * goff : GC * NB * (goff + ngr)]
                )
                w4_mt.append((goff, wt))
                xt_mt.append((goff, xt_))
                goff += ngr

            def w4v(g):
                for off, wt in reversed(w4_mt):
                    if g >= off:
                        return wt[:, GW * (g - off) : GW * (g - off + 1)]

            def xtv(k):
                g, j = k // GC, k % GC
                for off, xt_ in reversed(xt_mt):
                    if g >= off:
                        return xt_[:, NB * (GC * (g - off) + j) : NB * (GC * (g - off) + j) + NB]

            # route-shard inputs + consts
            xtl = sb.tile([128, NLC * B], BF16, tag="xtl", name="xtl")
            nc.scalar.dma_start(xtl[:], d_xtl[:])
            w4l = sb.tile([128, NLC * CO], BF16, tag="w4l", name="w4l")
            nc.scalar.dma_start(w4l[:], d_w4l[:])
            xr_t = []
            for h in range(2):
                xh = sb.tile([128, RL * I], BF16, tag=f"xr{h}", name=f"xr{h}")
                nc.scalar.dma_start(xh[:], d_xr[128 * h : 128 * h + 128, :])
                xr_t.append(xh)
            w4s = sb.tile([128, NLC * CO], BF16, tag="w4s")
            nc.scalar.dma_start(w4s[:], d_w4s[:])
            identq = const.tile([QP, QP], BF16, tag="identq")
            sel16 = const.tile([16, 128], BF16, tag="sel16")
            sel10 = const.tile([QP, 16], BF16, tag="sel10")
            ones1 = const.tile([1, 128], BF16, tag="ones1")
            bones = const.tile([128, 16], BF16, tag="bones")
            nc.scalar.dma_start(sel16[:], d_s16[:])
            nc.scalar.dma_start(sel10[:], d_s10[:])
            nc.scalar.dma_start(ones1[:], d_on[:])
            nc.scalar.dma_start(bones[:], d_bo[:])
            nc.scalar.dma_start(identq[:], d_idq[:])

            # ---- iter 0: batch-sharded s0 over all routes (c uniform = 1/R)
            s0_ps = pbig.tile([128, 2 * PSB], F32, tag="gbig")
            for k in range(NCHUNK):
                nc.tensor.matmul(
                    s0_ps[:NB, :CO],
                    lhsT=xtv(k),
                    rhs=w4v(k // GC)[:, CO * (k % GC) : CO * (k % GC) + CO],
                    start=(k == 0),
                    stop=(k == NCHUNK - 1),
                )
            v0 = _squash_tile(tc, pools, s0_ps[:NB, :CO], NB, 1.0 / R, None, "0")
            vb0 = sb.tile([NB, CO], BF16, tag="vb0", name="vb0")
            nc.vector.tensor_scalar_mul(vb0[:], v0[:], 1.0)
            ccv_in = dram.tile([NB, CO], BF16, tag="ccvin", name="ccv_in")
            ccv_out = dram.tile(
                [B, CO], BF16, tag="ccvout", addr_space="Shared", name="ccv_out"
            )
            nc.sync.dma_start(ccv_in[:], vb0[:])
            nc.gpsimd.collective_compute(
                "AllGather",
                ALU.bypass,
                replica_groups=[list(range(NCORES))],
                ins=[ccv_in[:].opt()],
                outs=[ccv_out[:].opt()],
            )
            vh0 = []
            for h in range(2):
                vt = sb.tile([128, CO], BF16, tag=f"vh{h}", bufs=2, name=f"vh{h}")
                nc.sync.dma_start(vt[:], ccv_out[128 * h : 128 * h + 128, :])
                vh0.append(vt)

            # ---- round 1: ab from v0 -> b1 -> Wc1 -> s1 (+Z1) -> AR -> v1
            q1 = _ab_q(tc, pools, xr_t, w4s, vh0, bones, tag=0)
            b_q1 = sb.tile([NLC * C, 16], F32, tag="b_q1")
            nc.vector.tensor_scalar_mul(b_q1[:], q1[:], 1.0)
            zl1 = sb.tile([NLC * C, 1], BF16, tag="zl_b", bufs=2, name="zl1")
            wc1 = wcp.tile([128, NLC * CO], BF16, tag="wc", name="wc1")
            _wc_from_bq(tc, pools, b_q1, w4l, identq, sel16, wc1, zl1, tag=1)
            sh1, zin1 = _s_round(tc, pools, xtl, wc1[:], zl1, sel10, tag=1)
            zb1 = _zb_build(tc, pools, zin1, identq, ones1, tag=1)
            vb1 = []
            for h in range(2):
                vf = _squash_tile(
                    tc, pools, sh1[h][:], 128, 1.0, zb1, f"1{h}"
                )
                vbh = sb.tile([128, CO], BF16, tag=f"vbh{h}", bufs=2, name=f"vb1{h}")
                nc.vector.tensor_scalar_mul(vbh[:], vf[:], 1.0)
                vb1.append(vbh)

            # ---- round 2 (final): b2 = b1 + ab(v1) -> Wc2 -> s2 (+Z2) -> AR
            q2 = _ab_q(tc, pools, xr_t, w4s, vb1, bones, tag=1)
            b_q2 = sb.tile([NLC * C, 16], F32, tag="b_q2")
            nc.vector.tensor_add(b_q2[:], b_q1[:], q2[:])
            zl2 = sb.tile([NLC * C, 1], BF16, tag="zl_b", bufs=2, name="zl2")
            wc2 = wcp.tile([128, NLC * CO], BF16, tag="wc", name="wc2")
            _wc_from_bq(tc, pools, b_q2, w4l, identq, sel16, wc2, zl2, tag=2)
            sh2, zin2 = _s_round(tc, pools, xtl, wc2[:], zl2, sel10, tag=2)
            zb2 = _zb_build(tc, pools, zin2, identq, ones1, tag=2)
            for h in range(2):
                vf = _squash_tile(
                    tc, pools, sh2[h][:], 128, 1.0, zb2, f"2{h}"
                )
                nc.sync.dma_start(d_out[128 * h : 128 * h + 128, :], vf[:])

    nc.compile()
    return nc


def _host_inputs(x, W):
    """Per-core input maps with pre-arranged layouts."""
    import ml_dtypes

    bf16 = ml_dtypes.bfloat16
    x = np.asarray(x, dtype=np.float32)
    W = np.asarray(W, dtype=np.float32)
    # W4[(r,i), (o,c)] chunk-major for the s_j moving operands
    wf_oc = np.ascontiguousarray(W.transpose(0, 3, 2, 1)).reshape(RI, CO)
    wfc_oc = wf_oc.reshape(NCHUNK, 128, CO)
    w4h = np.ascontiguousarray(wfc_oc.transpose(1, 0, 2)).reshape(
        128, NCHUNK * CO
    ).astype(bf16)
    # W4[(r,i), (c,o)] local chunks for the P-phase
    wf_co = np.ascontiguousarray(W.transpose(0, 3, 1, 2)).reshape(RI, CO)
    wfc_co = wf_co.reshape(NCHUNK, 128, CO)
    identq = np.eye(QP, dtype=bf16)
    q16, p16 = np.meshgrid(np.arange(16), np.arange(128), indexing="ij")
    sel16 = (q16 == p16 // 8).astype(bf16)
    # sel10[10a+c, c] = 1 (sums over a); top [10,10] block is identity
    qq, cc = np.meshgrid(np.arange(QP), np.arange(16), indexing="ij")
    sel10 = ((qq < NLC * C) & (qq % C == cc)).astype(bf16)
    ones1 = np.ones((1, 128), dtype=bf16)
    pq, jq = np.meshgrid(np.arange(128), np.arange(16), indexing="ij")
    bones = (pq // 8 == jq).astype(bf16)
    xf = x.reshape(B, RI)
    in_maps = []
    for c in range(NCORES):
        # iter-0 batch shard: own 32 batches, all 9216 (r,i), chunk-major
        xc = np.ascontiguousarray(xf[NB * c : NB * (c + 1)])
        xt = np.ascontiguousarray(xc.T)  # [9216, 32]
        xth = np.ascontiguousarray(
            xt.reshape(NCHUNK, 128, NB).transpose(1, 0, 2)
        ).reshape(128, NCHUNK * NB).astype(bf16)
        # route shard: local 1152 (r,i) rows, all 256 batches
        lo, hi = RL * I * c, RL * I * (c + 1)
        xl = np.ascontiguousarray(xf[:, lo:hi].T)  # [1152, 256]
        xtlh = np.ascontiguousarray(
            xl.reshape(NLC, 128, B).transpose(1, 0, 2)
        ).reshape(128, NLC * B).astype(bf16)
        xrh = xf[:, lo:hi].astype(bf16)
        w4lh = np.ascontiguousarray(
            wfc_oc[NLC * c : NLC * (c + 1)].transpose(1, 0, 2)
        ).reshape(128, NLC * CO).astype(bf16)
        w4sh = np.ascontiguousarray(
            wfc_co[NLC * c : NLC * (c + 1)].transpose(1, 0, 2)
        ).reshape(128, NLC * CO).astype(bf16)
        in_maps.append(
            {
                "xt": xth,
                "w4": w4h,
                "xtl": xtlh,
                "xr": xrh,
                "w4l": w4lh,
                "w4s": w4sh,
                "identq": identq,
                "sel16": sel16,
                "sel10": sel10,
                "ones1": ones1,
                "bones": bones,
            }
        )
    return in_maps


def kernel(x, W, trace=False):
    global _BUILT
    if _BUILT is None:
        _BUILT = build()
    nc = _BUILT
    in_maps = _host_inputs(x, W)
    res = bass_utils.run_bass_kernel_spmd(
        nc, in_maps, core_ids=list(range(NCORES)), trace=trace
    )
    v = np.concatenate(
        [res.results[c]["vout"][NB * c : NB * (c + 1)] for c in range(NCORES)], axis=0
    )
    out = v.reshape(B, C, O, 1)
    if trace:
        kernel.last_exec_time_ns = res.exec_time_ns
        kernel.last_results = res
    return out
